# revision 29
# baseline (speedup 1.0000x reference)
"""Deformable-DETR encoder (2 layers) fully on 8 Trainium2 NeuronCores.

Sharding: 8 cores = 2 batch x 4 query-bands, one SPMD NEFF for both
layers. Cross-core exchange (full x for each layer's value projection)
is an on-device AllGather within each 4-core replica group.

Per core/layer:
  - value = x @ Wv staged into a zero-padded, transposed bf16 grid
    VT[(head, dh-lane) partition, padded-pos, dh-pair] -- the padding
    implements grid_sample zero-padding for free.
  - per 128-query chunk: off/attn projections (PE, bf16), softmax over
    the 16 (level,point) slots via a block-ones matmul (avoids
    partition reductions), bilinear indices/weights on DVE (exact floor
    via the 2^23 magic-add + is_gt correction), 4 gpsimd ap_gathers
    whose index wrap (s%16 = level*4+point) matches the [(h,lp), q]
    compute layout exactly, per-dh-lane weight replication via a
    stride-0-source DMA bounce through DRAM, fused multiply +
    reduce-over-(level,point) on DVE, then W_out / LN / FFN / LN
    streaming per chunk (residual stream in fp32).

Host I/O is slimmed for the axon tunnel: bf16 band inputs + compact
per-level ref tables expanded on device by broadcast DMA, bf16 output,
weights cached device-side across calls by a persistent jit runner.

kernel(**inputs) takes FULL inputs, returns FULL [2, 13294, 256] fp32.
Retries the device path once, then falls back to a numpy
implementation, so the output is always correct.
"""
import numpy as np

NUM_LAYERS = 2
SHAPES = [(100, 100), (50, 50), (25, 25), (13, 13)]
D, NH, NP, NL = 256, 8, 4, 4
DH = D // NH
DFF = 1024
B = 2
S = sum(h * w for h, w in SHAPES)      # 13294
QB = 3328                               # band width (4 bands; last has 3310 valid)
CH = 128                                # phase-B query chunk
NCHUNK = QB // CH                       # 26
PX, PY = 4, 3                           # pad margins (left/top; right=3, bottom=3)
WP = [w + 7 for (h, w) in SHAPES]       # [107, 57, 32, 20]
HP = [h + 6 for (h, w) in SHAPES]       # [106, 56, 31, 19]
LBASE = [0]
for l in range(1, NL):
    LBASE.append(LBASE[-1] + HP[l - 1] * WP[l - 1])
NPOS = LBASE[-1] + HP[-1] * WP[-1]      # 15906
LSTART = [0, 10000, 12500, 13125]
BANDS = [(b * QB, min((b + 1) * QB, S)) for b in range(4)]
MAGIC = 12582912.0                      # 1.5 * 2^23: fp32 round-to-int trick
f32 = np.float32

_CACHE = {}


def _value_chunks():
    """Row-aligned chunks over S for the value matmul: (qg0, n, level, row0, nrows)."""
    out = []
    for l, (H, W) in enumerate(SHAPES):
        rows_per = max(1, 500 // W)
        r = 0
        while r < H:
            nr = min(rows_per, H - r)
            out.append((LSTART[l] + r * W, nr * W, l, r, nr))
            r += nr
    return out


def _build_nc(stage="full"):
    import concourse.bacc as bacc
    import concourse.mybir as mybir
    from concourse.tile import TileContext
    from concourse.bass_types import AP
    from concourse import library_config

    dt = mybir.dt
    AF = mybir.ActivationFunctionType
    AL = mybir.AluOpType
    AX = mybir.AxisListType

    nc = bacc.Bacc("TRN2", num_devices=8)

    def ext(name, shape, d=dt.float32):
        return nc.dram_tensor(name, shape, d, kind="ExternalInput")

    xband0 = ext("xband0", [QB, D], dt.bfloat16)
    refc = ext("refc", [2, 16, QB])            # (xy, (l,p), q): ref*W-0.5+PAD
    boffp = ext("boffp", [NUM_LAYERS, 2, 128])  # b_off permuted (xy, (h,l,p))
    wvp = ext("wvp", [NUM_LAYERS, D, D], dt.bfloat16)
    bvp = ext("bvp", [NUM_LAYERS, 128, 2])
    wox = ext("wox", [NUM_LAYERS, D, 128], dt.bfloat16)
    woy = ext("woy", [NUM_LAYERS, D, 128], dt.bfloat16)
    wat = ext("wat", [NUM_LAYERS, D, 128], dt.bfloat16)
    bat = ext("bat", [NUM_LAYERS, 128, 1])
    wop = ext("wop", [NUM_LAYERS, D, D], dt.bfloat16)
    bop = ext("bop", [NUM_LAYERS, 128, 2])
    w1 = ext("w1", [NUM_LAYERS, D, DFF], dt.bfloat16)
    b1 = ext("b1", [NUM_LAYERS, 128, 8])
    w2 = ext("w2", [NUM_LAYERS, DFF, D], dt.bfloat16)
    b2 = ext("b2", [NUM_LAYERS, 128, 2])
    g1r = ext("g1r", [NUM_LAYERS, 128, D])
    b1r = ext("b1r", [NUM_LAYERS, 128, D])
    g2r = ext("g2r", [NUM_LAYERS, 128, D])
    b2r = ext("b2r", [NUM_LAYERS, 128, D])
    consts = ext("consts", [128, 8])      # WPL, LBASE, XMAX, YMAX
    identb = ext("identb", [128, 128], dt.bfloat16)
    identf = ext("identf", [128, 128])
    bones = ext("bones", [128, 128], dt.bfloat16)

    x0int = nc.dram_tensor("x0int", [QB, D], dt.bfloat16, kind="Internal")
    x0full = nc.dram_tensor("x0full", [4, QB, D], dt.bfloat16, kind="Internal")
    x1band = nc.dram_tensor("x1band", [QB, D], dt.float32, kind="Internal")
    x1full = nc.dram_tensor("x1full", [4, QB, D], dt.float32, kind="Internal")
    wdram = nc.dram_tensor("wdram", [NUM_LAYERS, NCHUNK, 128, 4 * CH],
                           dt.bfloat16, kind="Internal")
    yband = nc.dram_tensor("yband", [QB, D], dt.bfloat16, kind="ExternalOutput")
    ybq = nc.dram_tensor("ybq", [QB, D], dt.int8, kind="ExternalOutput")
    ysc = nc.dram_tensor("ysc", [QB, 1], dt.float32, kind="ExternalOutput")

    RG = [[0, 1, 2, 3], [4, 5, 6, 7]]
    VCH = _value_chunks()

    with TileContext(nc) as tc:
        with (
            tc.tile_pool(name="persist", bufs=1) as pp,
            tc.tile_pool(name="wts", bufs=1) as wp_,
            tc.tile_pool(name="io", bufs=2) as io,
            tc.tile_pool(name="wk", bufs=1) as wk,
            tc.tile_pool(name="gt", bufs=1) as gt,
            tc.tile_pool(name="ps", bufs=3, space="PSUM") as ps,
            tc.tile_pool(name="psv", bufs=1, space="PSUM") as psv,
            tc.tile_pool(name="pst", bufs=2, space="PSUM") as pstp,
        ):
            nc.gpsimd.load_library(library_config.ap_gather)

            # ---- persistent tiles ----
            vt = pp.tile([128, NPOS * 2], dt.bfloat16, name="vt")
            xtb = pp.tile([128, 2, QB], dt.bfloat16, name="xtb")
            tid_b = pp.tile([128, 128], dt.bfloat16, name="tid_b")
            tid_f = pp.tile([128, 128], dt.float32, name="tid_f")
            tones = pp.tile([128, 128], dt.bfloat16, name="tones")
            tcst = pp.tile([128, 8], dt.float32, name="tcst")
            nc.sync.dma_start(tid_b[:], identb.ap())
            nc.sync.dma_start(tid_f[:], identf.ap())
            nc.sync.dma_start(tones[:], bones.ap())
            nc.sync.dma_start(tcst[:], consts.ap())
            tbofx = pp.tile([128, NUM_LAYERS], dt.float32, name="tbofx")
            tbofy = pp.tile([128, NUM_LAYERS], dt.float32, name="tbofy")
            nc.sync.dma_start(tbofx[:], boffp.ap().rearrange("l a p -> p (l a)")[:, 0::2])
            nc.sync.dma_start(tbofy[:], boffp.ap().rearrange("l a p -> p (l a)")[:, 1::2])
            WPL = tcst[:, 0:1]
            LBC = tcst[:, 1:2]
            XMX = tcst[:, 2:3]
            YMX = tcst[:, 3:4]

            # per-layer weight tiles (reloaded between layers)
            twv = wp_.tile([128, 2, D], dt.bfloat16, name="twv")
            tbv = wp_.tile([128, 2], dt.float32, name="tbv")
            tox = wp_.tile([128, 2, 128], dt.bfloat16, name="tox")
            toy = wp_.tile([128, 2, 128], dt.bfloat16, name="toy")
            tat = wp_.tile([128, 2, 128], dt.bfloat16, name="tat")
            tba = wp_.tile([128, 1], dt.float32, name="tba")
            two = wp_.tile([128, 2, D], dt.bfloat16, name="two")
            tbo = wp_.tile([128, 2], dt.float32, name="tbo")
            tw1 = wp_.tile([128, 2, DFF], dt.bfloat16, name="tw1")
            tb1 = wp_.tile([128, 8], dt.float32, name="tb1")
            tw2 = wp_.tile([128, 8, D], dt.bfloat16, name="tw2")
            tb2 = wp_.tile([128, 2], dt.float32, name="tb2")
            tg1 = wp_.tile([128, D], dt.float32, name="tg1")
            tb1r = wp_.tile([128, D], dt.float32, name="tb1r")
            tg2 = wp_.tile([128, D], dt.float32, name="tg2")
            tb2r = wp_.tile([128, D], dt.float32, name="tb2r")

            def load_layer_weights(li):
                nc.sync.dma_start(twv[:], wvp.ap()[li].rearrange("(k p) m -> p k m", p=128))
                nc.sync.dma_start(tbv[:], bvp.ap()[li])
                nc.sync.dma_start(tox[:], wox.ap()[li].rearrange("(k p) m -> p k m", p=128))
                nc.sync.dma_start(toy[:], woy.ap()[li].rearrange("(k p) m -> p k m", p=128))
                nc.sync.dma_start(tat[:], wat.ap()[li].rearrange("(k p) m -> p k m", p=128))
                nc.sync.dma_start(tba[:], bat.ap()[li])
                nc.sync.dma_start(two[:], wop.ap()[li].rearrange("(k p) m -> p k m", p=128))
                nc.sync.dma_start(tbo[:], bop.ap()[li])
                nc.sync.dma_start(tw1[:], w1.ap()[li].rearrange("(k p) m -> p k m", p=128))
                nc.sync.dma_start(tb1[:], b1.ap()[li])
                nc.sync.dma_start(tw2[:], w2.ap()[li].rearrange("(k p) m -> p k m", p=128))
                nc.sync.dma_start(tb2[:], b2.ap()[li])
                nc.sync.dma_start(tg1[:], g1r.ap()[li])
                nc.sync.dma_start(tb1r[:], b1r.ap()[li])
                nc.sync.dma_start(tg2[:], g2r.ap()[li])
                nc.sync.dma_start(tb2r[:], b2r.ap()[li])

            def transpose_rows_to(dst, src_rows_ap, n, src_bf16=False):
                """src rows [n, 256] (DRAM ap) -> dst [128, 2, nt*128] bf16 cols 0..n."""
                nt = (n + 127) // 128
                if src_bf16:
                    xrb = io.tile([128, nt, D], dt.bfloat16, tag="xrowsb")
                    for t in range(nt):
                        rows = min(128, n - t * 128)
                        nc.sync.dma_start(xrb[:rows, t, :],
                                          src_rows_ap[t * 128:t * 128 + rows, :])
                else:
                    xr = io.tile([128, nt, D], dt.float32, tag="xrows")
                    for t in range(nt):
                        rows = min(128, n - t * 128)
                        nc.sync.dma_start(xr[:rows, t, :],
                                          src_rows_ap[t * 128:t * 128 + rows, :])
                    xrb = io.tile([128, nt, D], dt.bfloat16, tag="xrowsb")
                    nc.vector.tensor_copy(xrb[:], xr[:])
                for t in range(nt):
                    for k in range(2):
                        pst = pstp.tile([128, 128], dt.bfloat16, tag="pt")
                        nc.tensor.transpose(
                            pst[:], xrb[:, t, k * 128:(k + 1) * 128], tid_b[:])
                        nc.scalar.activation(
                            dst[:, k, t * 128:(t + 1) * 128], pst[:], AF.Identity)

            def build_vt(xsrc_ap, src_bf16=False):
                """Value projection into padded transposed bf16 grid."""
                nc.vector.memset(vt[:], 0.0)
                vt3 = vt[:].rearrange("p (n e) -> p n e", e=2)
                for (qg0, n, l, r0, nr) in VCH:
                    W = SHAPES[l][1]
                    xtc = wk.tile([128, 2, 512], dt.bfloat16, tag="xtc")
                    transpose_rows_to(xtc, xsrc_ap[qg0:qg0 + n, :], n, src_bf16)
                    for e in range(2):
                        pv = psv.tile([128, 512], dt.float32, tag="pv")
                        for k in range(2):
                            nc.tensor.matmul(
                                pv[:, :n], twv[:, k, e * 128:(e + 1) * 128],
                                xtc[:, k, :n], start=(k == 0), stop=(k == 1))
                        start = LBASE[l] + (r0 + PY) * WP[l] + PX
                        dst = vt3[:, :, e][:, start:start + nr * WP[l]] \
                            .rearrange("p (r x) -> p r x", x=WP[l])[:, :, :W]
                        nc.scalar.activation(
                            dst, pv[:, :n].rearrange("p (r x) -> p r x", x=W),
                            AF.Identity, bias=tbv[:, e:e + 1])

            def phase_b(li, xband_ap, out_ap, no_gather=False, no_samp=False,
                        no_head=False, in_bf16=False, out_bf16=False):
                """26 query chunks: MSDA + residual/LN + FFN + residual/LN."""
                for c in range(NCHUNK):
                    q0 = c * CH
                    o1 = wk.tile([128, 2, CH], dt.float32, tag="o1")
                    if no_head:
                        nc.vector.memset(o1[:], 0.01)
                    else:
                        macb = wk.tile([128, 2, CH], dt.bfloat16, tag="macb")
                        if no_samp:
                            nc.vector.memset(macb[:], 0.01)
                        else:
                            _msda_chunk(li, c, q0, macb, no_gather=no_gather)
                        # --- W_out projection -> out1T [128, 2, CH] fp32 ---
                        for m in range(2):
                            pw = ps.tile([128, CH], dt.float32, tag="mm")
                            for e in range(2):
                                nc.tensor.matmul(pw[:], two[:, e, m * 128:(m + 1) * 128],
                                                 macb[:, e, :], start=(e == 0), stop=(e == 1))
                            nc.scalar.activation(o1[:, m, :], pw[:], AF.Identity,
                                                 bias=tbo[:, m:m + 1])
                    # --- residual + LN1 (rows [128q, 256]) ---
                    xr = io.tile([128, D],
                                 dt.bfloat16 if in_bf16 else dt.float32,
                                 tag="xrb" if in_bf16 else "xr")
                    nc.sync.dma_start(xr[:], xband_ap[q0:q0 + CH, :])
                    h1 = wk.tile([128, D], dt.float32, tag="h1")
                    for m in range(2):
                        ptr = pstp.tile([128, 128], dt.float32, tag="ptf")
                        nc.tensor.transpose(ptr[:], o1[:, m, :], tid_f[:])
                        nc.vector.tensor_tensor(h1[:, m * 128:(m + 1) * 128],
                                                xr[:, m * 128:(m + 1) * 128],
                                                ptr[:], AL.add)
                    x1c = wk.tile([128, D], dt.float32, tag="x1c")
                    _layernorm(nc, wk, dt, AF, AL, AX, h1, x1c, tg1, tb1r)
                    # --- FFN ---
                    x1b = wk.tile([128, D], dt.bfloat16, tag="x1b")
                    nc.vector.tensor_copy(x1b[:], x1c[:])
                    x1t = wk.tile([128, 2, CH], dt.bfloat16, tag="x1t")
                    for k in range(2):
                        ptb = pstp.tile([128, 128], dt.bfloat16, tag="pt")
                        nc.tensor.transpose(ptb[:], x1b[:, k * 128:(k + 1) * 128], tid_b[:])
                        nc.scalar.activation(x1t[:, k, :], ptb[:], AF.Identity)
                    ht = wk.tile([128, 8, CH], dt.bfloat16, tag="ht")
                    for m in range(8):
                        ph = ps.tile([128, CH], dt.float32, tag="mm")
                        for k in range(2):
                            nc.tensor.matmul(ph[:], tw1[:, k, m * 128:(m + 1) * 128],
                                             x1t[:, k, :], start=(k == 0), stop=(k == 1))
                        nc.scalar.activation(ht[:, m, :], ph[:], AF.Relu,
                                             bias=tb1[:, m:m + 1])
                    y2 = wk.tile([128, 2, CH], dt.float32, tag="y2")
                    for m in range(2):
                        py = ps.tile([128, CH], dt.float32, tag="mm")
                        for k in range(8):
                            nc.tensor.matmul(py[:], tw2[:, k, m * 128:(m + 1) * 128],
                                             ht[:, k, :], start=(k == 0), stop=(k == 7))
                        nc.scalar.activation(y2[:, m, :], py[:], AF.Identity,
                                             bias=tb2[:, m:m + 1])
                    y2b = wk.tile([128, 2, CH], dt.bfloat16, tag="y2b")
                    nc.vector.tensor_copy(y2b[:], y2[:])
                    h2 = wk.tile([128, D], dt.float32, tag="h2")
                    for m in range(2):
                        pt2 = pstp.tile([128, 128], dt.bfloat16, tag="pt")
                        nc.tensor.transpose(pt2[:], y2b[:, m, :], tid_b[:])
                        nc.vector.tensor_tensor(h2[:, m * 128:(m + 1) * 128],
                                                x1c[:, m * 128:(m + 1) * 128],
                                                pt2[:], AL.add)
                    xout = io.tile([128, D], dt.float32, tag="xout")
                    _layernorm(nc, wk, dt, AF, AL, AX, h2, xout, tg2, tb2r)
                    if out_bf16:
                        xob = io.tile([128, D], dt.bfloat16, tag="xob")
                        nc.vector.tensor_copy(xob[:], xout[:])
                        nc.sync.dma_start(out_ap[q0:q0 + CH, :], xob[:])
                        amx = wk.tile([128, 1], dt.float32, tag="amx")
                        nc.vector.tensor_reduce(amx[:], xout[:], AX.X, AL.max,
                                                apply_absolute_value=True)
                        inv = wk.tile([128, 1], dt.float32, tag="ainv")
                        nc.vector.reciprocal(inv[:], amx[:])
                        qf = wk.tile([128, D], dt.float32, tag="qf")
                        nc.vector.tensor_scalar(qf[:], xout[:], inv[:], 127.0,
                                                AL.mult, AL.mult)
                        qi = io.tile([128, D], dt.int8, tag="qi")
                        nc.vector.tensor_copy(qi[:], qf[:])
                        nc.sync.dma_start(ybq.ap()[q0:q0 + CH, :], qi[:])
                        nc.sync.dma_start(ysc.ap()[q0:q0 + CH, :], amx[:])
                    else:
                        nc.sync.dma_start(out_ap[q0:q0 + CH, :], xout[:])

            def _msda_chunk(li, c, q0, macb, no_gather=False):
                # --- projections: offx/offy/attn (PSUM [128, CH]) ---
                pox = ps.tile([128, CH], dt.float32, tag="mm")
                poy = ps.tile([128, CH], dt.float32, tag="mm")
                pat = ps.tile([128, CH], dt.float32, tag="mm")
                for (pt, wt) in ((pox, tox), (poy, toy), (pat, tat)):
                    for k in range(2):
                        nc.tensor.matmul(pt[:], wt[:, k, :],
                                         xtb[:, k, q0:q0 + CH],
                                         start=(k == 0), stop=(k == 1))
                # --- sample coords (padded grid units) ---
                rx = wk.tile([128, CH], dt.float32, tag="rx")
                ry = wk.tile([128, CH], dt.float32, tag="ry")
                for (dst_t, xy) in ((rx, 0), (ry, 1)):
                    base = refc.ap()[xy]
                    rsrc = AP(base.tensor, base.offset + q0,
                              [[0, 8], [QB, 16], [1, CH]])
                    nc.sync.dma_start(dst_t[:], rsrc)
                xg = wk.tile([128, CH], dt.float32, tag="xg")
                yg = wk.tile([128, CH], dt.float32, tag="yg")
                nc.vector.tensor_scalar(xg[:], pox[:], tbofx[:, li:li + 1], None, AL.add)
                nc.vector.tensor_scalar(yg[:], poy[:], tbofy[:, li:li + 1], None, AL.add)
                nc.vector.tensor_tensor(xg[:], xg[:], rx[:], AL.add)
                nc.vector.tensor_tensor(yg[:], yg[:], ry[:], AL.add)
                # --- exact floor + frac ---
                x0 = wk.tile([128, CH], dt.float32, tag="x0")
                y0 = wk.tile([128, CH], dt.float32, tag="y0")
                fx = wk.tile([128, CH], dt.float32, tag="fx")
                fy = wk.tile([128, CH], dt.float32, tag="fy")
                for (g_, o_, f_) in ((xg, x0, fx), (yg, y0, fy)):
                    rnd = wk.tile([128, CH], dt.float32, tag="rnd")
                    nc.vector.tensor_scalar(rnd[:], g_[:], MAGIC, -MAGIC,
                                            AL.add, AL.add)
                    msk = wk.tile([128, CH], dt.float32, tag="msk")
                    nc.vector.tensor_tensor(msk[:], rnd[:], g_[:], AL.is_gt)
                    nc.vector.tensor_tensor(o_[:], rnd[:], msk[:], AL.subtract)
                    nc.vector.tensor_tensor(f_[:], g_[:], o_[:], AL.subtract)
                # clamp (keeps +1 corners inside each level block)
                nc.vector.tensor_scalar(x0[:], x0[:], 0.0, XMX, AL.max, AL.min)
                nc.vector.tensor_scalar(y0[:], y0[:], 0.0, YMX, AL.max, AL.min)
                # --- linear indices, 4 corners, int16 ---
                ib = wk.tile([128, CH], dt.float32, tag="ib")
                nc.vector.tensor_scalar(ib[:], y0[:], WPL, LBC, AL.mult, AL.add)
                nc.vector.tensor_tensor(ib[:], ib[:], x0[:], AL.add)
                ir = wk.tile([128, CH], dt.float32, tag="ir")
                nc.vector.tensor_scalar(ir[:], ib[:], WPL, None, AL.add)
                idx = []
                for (src_, off) in ((ib, 0.0), (ib, 1.0), (ir, 0.0), (ir, 1.0)):
                    ii = wk.tile([128, CH], dt.int16, tag=f"idx{len(idx)}")
                    if off == 0.0:
                        nc.vector.tensor_copy(ii[:], src_[:])
                    else:
                        tmp = wk.tile([128, CH], dt.float32, tag="itmp")
                        nc.vector.tensor_scalar(tmp[:], src_[:], off, None, AL.add)
                        nc.vector.tensor_copy(ii[:], tmp[:])
                    idx.append(ii)
                # --- softmax over 16 (l,p) per head ---
                ex = wk.tile([128, CH], dt.bfloat16, tag="ex")
                nc.scalar.activation(ex[:], pat[:], AF.Exp, bias=tba[:, 0:1])
                pse = ps.tile([128, CH], dt.float32, tag="mm")
                nc.tensor.matmul(pse[:], tones[:], ex[:], start=True, stop=True)
                rs = wk.tile([128, CH], dt.float32, tag="rs")
                nc.vector.reciprocal(rs[:], pse[:])
                aw = wk.tile([128, CH], dt.float32, tag="aw")
                nc.vector.tensor_tensor(aw[:], ex[:], rs[:], AL.mult)
                # --- corner weights -> wstack bf16 [128, 4, CH] ---
                wx0 = wk.tile([128, CH], dt.float32, tag="wx0")
                wy0 = wk.tile([128, CH], dt.float32, tag="wy0")
                nc.vector.tensor_scalar(wx0[:], fx[:], -1.0, 1.0, AL.mult, AL.add)
                nc.vector.tensor_scalar(wy0[:], fy[:], -1.0, 1.0, AL.mult, AL.add)
                u0 = wk.tile([128, CH], dt.float32, tag="u0")
                u1 = wk.tile([128, CH], dt.float32, tag="u1")
                nc.vector.tensor_tensor(u0[:], aw[:], wx0[:], AL.mult)
                nc.vector.tensor_tensor(u1[:], aw[:], fx[:], AL.mult)
                wst = wk.tile([128, 4, CH], dt.bfloat16, tag="wst")
                nc.vector.tensor_tensor(wst[:, 0, :], u0[:], wy0[:], AL.mult)
                nc.vector.tensor_tensor(wst[:, 1, :], u1[:], wy0[:], AL.mult)
                nc.vector.tensor_tensor(wst[:, 2, :], u0[:], fy[:], AL.mult)
                nc.vector.tensor_tensor(wst[:, 3, :], u1[:], fy[:], AL.mult)
                # --- replicate weights across 16 dh-lanes via DRAM bounce ---
                wslot = wdram.ap()[li][c]
                nc.sync.dma_start(wslot, wst[:].rearrange("p a q -> p (a q)"))
                wrep = wk.tile([128, 16, 4 * CH], dt.bfloat16, tag="wrep")
                rep_src = AP(wslot.tensor, wslot.offset,
                             [[16 * 4 * CH, 8], [0, 16], [4 * CH, 16],
                              [1, 4 * CH]])
                nc.sync.dma_start(wrep[:], rep_src)
                # --- 4 gathers + weighted reduce over (lp, corners) ---
                red = wk.tile([128, 8, CH], dt.float32, tag="red")
                for ci in range(4):
                    g = gt.tile([128, CH * 16, 2], dt.bfloat16, tag=f"g{ci}")
                    if no_gather:
                        nc.vector.memset(g[:], 0.25)
                    else:
                        nc.gpsimd.ap_gather(g[:], vt[:], idx[ci][:], channels=128,
                                            num_elems=NPOS, d=2, num_idxs=CH * 16)
                    prod = wk.tile([128, CH, 2, 16], dt.bfloat16, tag="prod")
                    w3 = wrep[:, :, ci * CH:(ci + 1) * CH] \
                        .rearrange("p l q -> p q l")
                    w4 = AP(w3.tensor, w3.offset,
                            [list(w3.ap[0]), list(w3.ap[1]), [0, 2],
                             list(w3.ap[2])])
                    nc.vector.tensor_tensor(
                        prod[:],
                        g[:].rearrange("p (q l) e -> p q e l", l=16),
                        w4, AL.mult)
                    nc.vector.tensor_reduce(
                        red[:, 2 * ci:2 * ci + 2, :].rearrange("p e q -> p q e"),
                        prod[:], AX.X, AL.add)
                mac = wk.tile([128, 2, CH], dt.float32, tag="mac")
                nc.vector.tensor_tensor(mac[:], red[:, 0:2, :], red[:, 2:4, :], AL.add)
                nc.vector.tensor_tensor(mac[:], mac[:], red[:, 4:6, :], AL.add)
                nc.vector.tensor_tensor(mac[:], mac[:], red[:, 6:8, :], AL.add)
                nc.vector.tensor_copy(macb[:], mac[:])

            # ================= stage x0 + exchange =================
            for c in range(NCHUNK // 2):
                r0, r1 = 2 * c * CH, 2 * (c + 1) * CH
                stg = io.tile([128, 2, D], dt.bfloat16, tag="xstage")
                nc.sync.dma_start(stg[:], xband0.ap()[r0:r1, :]
                                  .rearrange("(t p) d -> p t d", p=128))
                nc.sync.dma_start(x0int.ap()[r0:r1, :]
                                  .rearrange("(t p) d -> p t d", p=128), stg[:])
            nc.gpsimd.collective_compute(
                "AllGather", mybir.AluOpType.bypass, RG,
                ins=[x0int.ap()], outs=[x0full.ap()])

            # ================= layer 0 =================
            load_layer_weights(0)
            build_vt(x0full.ap().rearrange("b q d -> (b q) d"), src_bf16=True)
            for c in range(NCHUNK):
                xtc2 = wk.tile([128, 2, 512], dt.bfloat16, tag="xtc")
                transpose_rows_to(xtc2, xband0.ap()[c * CH:(c + 1) * CH, :], CH,
                                  src_bf16=True)
                nc.vector.tensor_copy(xtb[:, :, c * CH:(c + 1) * CH], xtc2[:, :, :CH])
            if stage == "a":
                dump = wk.tile([128, 2048], dt.float32, tag="dump")
                nc.vector.tensor_copy(dump[:], vt[:, :2048])
                nc.sync.dma_start(
                    yband.ap().rearrange("q d -> (q d)")[0:262144]
                    .rearrange("(p f) -> p f", p=128), dump[:])
            else:
                phase_b(0, xband0.ap(), x1band.ap() if stage == "full" else yband.ap(),
                        no_gather=(stage == "bng"),
                        no_samp=(stage == "b_mid"),
                        no_head=(stage == "b_tail"), in_bf16=True,
                        out_bf16=(stage != "full"))

            if stage == "full":
                # ================= exchange =================
                nc.gpsimd.collective_compute(
                    "AllGather", mybir.AluOpType.bypass, RG,
                    ins=[x1band.ap()], outs=[x1full.ap()])

                # ================= layer 1 =================
                load_layer_weights(1)
                build_vt(x1full.ap().rearrange("b q d -> (b q) d"))
                for c in range(NCHUNK):
                    xtc3 = wk.tile([128, 2, 512], dt.bfloat16, tag="xtc")
                    transpose_rows_to(xtc3, x1band.ap()[c * CH:(c + 1) * CH, :], CH)
                    nc.vector.tensor_copy(xtb[:, :, c * CH:(c + 1) * CH], xtc3[:, :, :CH])
                phase_b(1, x1band.ap(), yband.ap(), out_bf16=True)

    nc.finalize()
    return nc


def _layernorm(nc, wk, dt, AF, AL, AX, hin, hout, gtile, btile):
    """Row layernorm [128 tokens, 256], eps=1e-5, with replicated g/b tiles."""
    sm = wk.tile([128, 1], dt.float32, tag="ln_sm")
    nc.vector.tensor_reduce(sm[:], hin[:], AX.X, AL.add)
    scr = wk.tile([128, 256], dt.float32, tag="ln_scr")
    nc.vector.tensor_tensor(scr[:], hin[:], hin[:], AL.mult)
    sq = wk.tile([128, 1], dt.float32, tag="ln_sq")
    nc.vector.tensor_reduce(sq[:], scr[:], AX.X, AL.add)
    nc.vector.tensor_scalar(sq[:], sq[:], 1.0 / 256, None, AL.mult)
    m = wk.tile([128, 1], dt.float32, tag="ln_m")
    nc.vector.tensor_scalar(m[:], sm[:], 1.0 / 256, None, AL.mult)
    mm2 = wk.tile([128, 1], dt.float32, tag="ln_mm")
    nc.vector.tensor_tensor(mm2[:], m[:], m[:], AL.mult)
    var = wk.tile([128, 1], dt.float32, tag="ln_v")
    nc.vector.tensor_tensor(var[:], sq[:], mm2[:], AL.subtract)
    nc.vector.tensor_scalar(var[:], var[:], 1e-5, None, AL.add)
    std = wk.tile([128, 1], dt.float32, tag="ln_s")
    nc.scalar.activation(std[:], var[:], AF.Sqrt)
    rstd = wk.tile([128, 1], dt.float32, tag="ln_r")
    nc.vector.reciprocal(rstd[:], std[:])
    xh = wk.tile([128, 256], dt.float32, tag="ln_xh")
    nc.vector.tensor_scalar(xh[:], hin[:], m[:], rstd[:], AL.subtract, AL.mult)
    nc.vector.tensor_tensor(xh[:], xh[:], gtile[:], AL.mult)
    nc.vector.tensor_tensor(hout[:], xh[:], btile[:], AL.add)


def _host_prep(src, spatial_shapes, valid_ratios, W_off, b_off, W_attn, b_attn,
               W_val, b_val, W_out, b_out, ln1_g, ln1_b, W1, b1, W2, b2,
               ln2_g, ln2_b):
    """Build per-core in_maps (weights permuted to device layouts)."""
    import ml_dtypes
    bf = ml_dtypes.bfloat16
    L = NUM_LAYERS

    # reference points (exact reference formula, incl. valid_ratios)
    vr = np.asarray(valid_ratios, f32)           # [B, NL, 2]
    refs = []
    for lvl, (H_, W_) in enumerate(SHAPES):
        ry, rx = np.meshgrid(np.linspace(0.5, H_ - 0.5, H_, dtype=f32),
                             np.linspace(0.5, W_ - 0.5, W_, dtype=f32),
                             indexing='ij')
        ry = ry.reshape(-1)[None] / (vr[:, None, lvl, 1] * H_)
        rx = rx.reshape(-1)[None] / (vr[:, None, lvl, 0] * W_)
        refs.append(np.stack([rx, ry], -1))
    ref = np.concatenate(refs, 1)                 # [B, S, 2]
    refl = ref[:, :, None] * vr[:, None]          # [B, S, NL, 2]

    # partition maps: p = 16h + 4l + pp
    hh = np.arange(128) // 16
    ll = (np.arange(128) % 16) // 4
    pp_ = np.arange(128) % 4
    Wl = np.array([SHAPES[l][1] for l in range(NL)], f32)
    Hl = np.array([SHAPES[l][0] for l in range(NL)], f32)

    # per (layer, batch): REFB tiles [128, QB] per band
    off_cols_x = ((hh * NL + ll) * NP + pp_) * 2
    off_cols_y = off_cols_x + 1
    # compact per-core ref tiles [2(xy), NL, QB] and permuted offset biases
    refc_all = {}
    for bi in range(B):
        gx = refl[bi, :, :, 0] * Wl[None, :] - 0.5 + PX    # [S, NL]
        gy = refl[bi, :, :, 1] * Hl[None, :] - 0.5 + PY
        for bd in range(4):
            a, bnd = BANDS[bd]
            t = np.full((2, 16, QB), 10.0, f32)
            t[0, :, :bnd - a] = np.repeat(gx[a:bnd].T, 4, axis=0)
            t[1, :, :bnd - a] = np.repeat(gy[a:bnd].T, 4, axis=0)
            refc_all[(bi, bd)] = t
    boffp = np.zeros((L, 2, 128), f32)
    for li in range(L):
        boffp[li, 0] = np.asarray(b_off[li], f32)[off_cols_x]
        boffp[li, 1] = np.asarray(b_off[li], f32)[off_cols_y]

    # weight permutations (same for every core)
    j16 = np.arange(128) % 16
    h8 = np.arange(128) // 16
    wvp = np.zeros((L, D, D), bf)
    bvp = np.zeros((L, 128, 2), f32)
    wop = np.zeros((L, D, D), bf)
    for li in range(L):
        for e in range(2):
            cols = h8 * 32 + 16 * e + j16          # dh for partition (h,j), plane e
            wvp[li, :, e * 128:(e + 1) * 128] = np.asarray(W_val[li], f32)[:, cols].astype(bf)
            bvp[li, :, e] = np.asarray(b_val[li], f32)[cols]
            wop[li, e * 128:(e + 1) * 128, :] = np.asarray(W_out[li], f32)[cols, :].astype(bf)
    wox = np.stack([np.asarray(W_off[li], f32)[:, off_cols_x].astype(bf) for li in range(L)])
    woy = np.stack([np.asarray(W_off[li], f32)[:, off_cols_y].astype(bf) for li in range(L)])
    wat = np.stack([np.asarray(W_attn[li], f32).astype(bf) for li in range(L)])
    bat = np.stack([np.asarray(b_attn[li], f32)[:, None] for li in range(L)])
    bop = np.stack([np.asarray(b_out[li], f32).reshape(2, 128).T for li in range(L)])
    w1s = np.stack([np.asarray(W1[li], f32).astype(bf) for li in range(L)])
    b1s = np.stack([np.asarray(b1[li], f32).reshape(8, 128).T for li in range(L)])
    w2s = np.stack([np.asarray(W2[li], f32).astype(bf) for li in range(L)])
    b2s = np.stack([np.asarray(b2[li], f32).reshape(2, 128).T for li in range(L)])
    g1r = np.stack([np.tile(np.asarray(ln1_g[li], f32), (128, 1)) for li in range(L)])
    b1r = np.stack([np.tile(np.asarray(ln1_b[li], f32), (128, 1)) for li in range(L)])
    g2r = np.stack([np.tile(np.asarray(ln2_g[li], f32), (128, 1)) for li in range(L)])
    b2r = np.stack([np.tile(np.asarray(ln2_b[li], f32), (128, 1)) for li in range(L)])

    consts = np.zeros((128, 8), f32)
    consts[:, 0] = np.array(WP, f32)[ll]
    consts[:, 1] = np.array(LBASE, f32)[ll]
    consts[:, 2] = np.array(WP, f32)[ll] - 2
    consts[:, 3] = np.array(HP, f32)[ll] - 2
    identb = np.eye(128, dtype=bf)
    identf = np.eye(128, dtype=f32)
    bones = (np.arange(128)[:, None] // 16 == np.arange(128)[None, :] // 16).astype(bf)

    srcf = np.asarray(src, f32)
    in_maps = []
    for core in range(8):
        bi, bd = core // 4, core % 4
        a, bnd = BANDS[bd]
        xband = np.zeros((QB, D), bf)
        xband[:bnd - a] = srcf[bi, a:bnd].astype(bf)
        in_maps.append({
            "xband0": xband,
            "refc": refc_all[(bi, bd)], "boffp": boffp,
            "wvp": wvp, "bvp": bvp, "wox": wox, "woy": woy, "wat": wat,
            "bat": bat, "wop": wop, "bop": bop, "w1": w1s, "b1": b1s,
            "w2": w2s, "b2": b2s, "g1r": g1r, "b1r": b1r, "g2r": g2r,
            "b2r": b2r, "consts": consts, "identb": identb,
            "identf": identf, "bones": bones,
        })
    return in_maps


class _Runner:
    """Persistent jit wrapper around the bass NEFF (trace once, reuse)."""

    def __init__(self, nc, n_cores=8):
        import jax
        import concourse.mybir as mybir
        from concourse import bass2jax
        from jax.sharding import Mesh, PartitionSpec
        from jax.experimental.shard_map import shard_map

        bass2jax.install_neuronx_cc_hook()
        self.n_cores = n_cores
        partition_name = (nc.partition_id_tensor.name
                          if nc.partition_id_tensor else None)
        in_names, out_names, out_avals, zero_shapes = [], [], [], []
        for alloc in nc.m.functions[0].allocations:
            if not isinstance(alloc, mybir.MemoryLocationSet):
                continue
            name = alloc.memorylocations[0].name
            if alloc.kind == "ExternalInput":
                if name != partition_name:
                    in_names.append(name)
            elif alloc.kind == "ExternalOutput":
                shape = tuple(alloc.tensor_shape)
                dtype = mybir.dt.np(alloc.dtype)
                out_names.append(name)
                out_avals.append(jax.core.ShapedArray(shape, dtype))
                zero_shapes.append((shape, dtype))
        self.in_names = list(in_names)
        self.out_names = out_names
        self.out_avals = out_avals
        self.zero_shapes = zero_shapes
        n_params = len(in_names)
        donate = ()
        all_names = in_names + out_names
        if partition_name is not None:
            all_names.append(partition_name)

        def _body(*args):
            operands = list(args)
            if partition_name is not None:
                operands.append(bass2jax.partition_id_tensor())
            outs = bass2jax._bass_exec_p.bind(
                *operands, out_avals=tuple(out_avals),
                in_names=tuple(all_names), out_names=tuple(out_names),
                lowering_input_output_aliases=(),
                sim_require_finite=True, sim_require_nnan=True, nc=nc)
            return tuple(outs)

        devices = jax.devices()[:n_cores]
        mesh = Mesh(np.asarray(devices), ("core",))
        self.sharding = jax.sharding.NamedSharding(mesh, PartitionSpec("core"))
        in_specs = (PartitionSpec("core"),) * (n_params + len(out_names))
        out_specs = (PartitionSpec("core"),) * len(out_names)
        self.jf = jax.jit(
            shard_map(_body, mesh=mesh, in_specs=in_specs,
                      out_specs=out_specs, check_rep=False),
            donate_argnums=donate, keep_unused=True)
        import jax.numpy as jnp

        def _mkzeros():
            return tuple(jnp.zeros((n_cores * s[0], *s[1:]), d)
                         for (s, d) in self.zero_shapes)
        self.zf = jax.jit(_mkzeros,
                          out_shardings=(self.sharding,) * len(out_names))
        self._zeros = None
        self._dev_cache = {}

    def __call__(self, in_maps):
        import jax
        n = self.n_cores
        concat_in = []
        for name in self.in_names:
            arrs = [np.asarray(in_maps[c][name]) for c in range(n)]
            key = tuple(id(a) for a in arrs)
            hit = self._dev_cache.get(name)
            if hit is not None and hit[0] == key:
                concat_in.append(hit[1])
            else:
                dev = jax.device_put(np.concatenate(arrs, axis=0),
                                     self.sharding)
                self._dev_cache[name] = (key, dev)
                concat_in.append(dev)
        if self._zeros is None:
            self._zeros = self.zf()
        out_arrs = self.jf(*concat_in, *self._zeros)
        jax.block_until_ready(out_arrs)
        fetch = getattr(self, "fetch_names", None) or self.out_names
        res = [dict() for _ in range(n)]
        for i, name in enumerate(self.out_names):
            if name not in fetch:
                continue
            arr = np.asarray(out_arrs[i]).reshape(n, *self.out_avals[i].shape)
            for c in range(n):
                res[c][name] = arr[c]
        return res


USE_INT8_OUT = True


def run_device(in_maps):
    if "runner" not in _CACHE:
        if "nc" not in _CACHE:
            _CACHE["nc"] = _build_nc()
        _CACHE["runner"] = _Runner(_CACHE["nc"])
        _CACHE["runner"].fetch_names = (
            ["ybq", "ysc"] if USE_INT8_OUT else ["yband"])
    results = _CACHE["runner"](in_maps)
    out = np.zeros((B, S, D), f32)
    for core in range(8):
        bi, bd = core // 4, core % 4
        a, bnd = BANDS[bd]
        r = results[core]
        if USE_INT8_OUT:
            q = r["ybq"][:bnd - a].astype(f32)
            s = r["ysc"][:bnd - a].astype(f32) * (1.0 / 127.0)
            out[bi, a:bnd] = q * s
        else:
            out[bi, a:bnd] = r["yband"][:bnd - a].astype(f32)
    return out


# ---------------- numpy fallback (correctness insurance) ----------------

def _np_layer_norm(x, g, b, eps=1e-5):
    m = x.mean(-1, keepdims=True, dtype=f32)
    v = x.var(-1, keepdims=True)
    return ((x - m) / np.sqrt(v + eps) * g + b).astype(f32)


def _np_softmax(x):
    m = x.max(-1, keepdims=True)
    e = np.exp(x - m)
    return (e / e.sum(-1, keepdims=True)).astype(f32)


def _np_msda(x, refl, Wv, bv, Wo, bo, Wa, ba, Wout, bout):
    value = (x @ Wv + bv).reshape(S, NH, DH)
    off = (x @ Wo + bo).reshape(S, NH, NL, NP, 2)
    attn = _np_softmax((x @ Wa + ba).reshape(S, NH, NL * NP)).reshape(S, NH, NL, NP)
    h_br = np.arange(NH, dtype=np.int32)[None, :, None]
    out = np.zeros((S, NH, DH), f32)
    start = 0
    PAD = 4
    for l, (H_, W_) in enumerate(SHAPES):
        Hp_, Wp_ = H_ + 2 * PAD, W_ + 2 * PAD
        vp = np.zeros((Hp_, Wp_, NH, DH), f32)
        vp[PAD:PAD + H_, PAD:PAD + W_] = value[start:start + H_ * W_].reshape(H_, W_, NH, DH)
        vp = vp.reshape(Hp_ * Wp_, NH, DH)
        xg_ = refl[:, l, 0][:, None, None] * W_ - 0.5 + off[:, :, l, :, 0] + PAD
        yg_ = refl[:, l, 1][:, None, None] * H_ - 0.5 + off[:, :, l, :, 1] + PAD
        x0 = np.floor(xg_)
        y0 = np.floor(yg_)
        fx = (xg_ - x0).astype(f32)
        fy = (yg_ - y0).astype(f32)
        i0 = (np.clip(y0, 0, Hp_ - 2) * Wp_ + np.clip(x0, 0, Wp_ - 2)).astype(np.int32)
        a_l = attn[:, :, l]
        for didx, w in ((0, (1 - fx) * (1 - fy)), (1, fx * (1 - fy)),
                        (Wp_, (1 - fx) * fy), (Wp_ + 1, fx * fy)):
            g = vp[i0 + didx, h_br]
            out += np.einsum('qhpd,qhp->qhd', g, (w * a_l).astype(f32))
        start += H_ * W_
    return (out.reshape(S, D) @ Wout + bout).astype(f32)


def _np_kernel(src, valid_ratios, W_off, b_off, W_attn, b_attn, W_val, b_val,
               W_out, b_out, ln1_g, ln1_b, W1, b1, W2, b2, ln2_g, ln2_b):
    vr = np.asarray(valid_ratios, f32)
    refs = []
    for lvl, (H_, W_) in enumerate(SHAPES):
        ry, rx = np.meshgrid(np.linspace(0.5, H_ - 0.5, H_, dtype=f32),
                             np.linspace(0.5, W_ - 0.5, W_, dtype=f32), indexing='ij')
        ry = ry.reshape(-1)[None] / (vr[:, None, lvl, 1] * H_)
        rx = rx.reshape(-1)[None] / (vr[:, None, lvl, 0] * W_)
        refs.append(np.stack([rx, ry], -1))
    ref = np.concatenate(refs, 1)
    refl = ref[:, :, None] * vr[:, None]
    x = np.asarray(src, f32).copy()
    for i in range(NUM_LAYERS):
        for bi in range(B):
            x2 = _np_msda(x[bi], refl[bi], W_val[i], b_val[i], W_off[i], b_off[i],
                          W_attn[i], b_attn[i], W_out[i], b_out[i])
            xb = _np_layer_norm(x[bi] + x2, ln1_g[i], ln1_b[i])
            h = np.maximum(xb @ W1[i] + b1[i], 0) @ W2[i] + b2[i]
            x[bi] = _np_layer_norm(xb + h, ln2_g[i], ln2_b[i])
    return x.astype(f32)


def kernel(src, spatial_shapes, valid_ratios, W_off, b_off, W_attn, b_attn,
           W_val, b_val, W_out, b_out, ln1_g, ln1_b, W1, b1, W2, b2,
           ln2_g, ln2_b):
    args = dict(src=src, spatial_shapes=spatial_shapes, valid_ratios=valid_ratios,
                W_off=W_off, b_off=b_off, W_attn=W_attn, b_attn=b_attn,
                W_val=W_val, b_val=b_val, W_out=W_out, b_out=b_out,
                ln1_g=ln1_g, ln1_b=ln1_b, W1=W1, b1=b1, W2=W2, b2=b2,
                ln2_g=ln2_g, ln2_b=ln2_b)
    try:
        in_maps = _host_prep(**args)
    except Exception:
        import traceback
        traceback.print_exc()
        a2 = dict(args)
        a2.pop("spatial_shapes")
        return _np_kernel(**a2)
    for attempt in range(2):
        try:
            return run_device(in_maps)
        except Exception:
            import traceback
            traceback.print_exc()
            _CACHE.pop("runner", None)
    a2 = dict(args)
    a2.pop("spatial_shapes")
    return _np_kernel(**a2)


# revision 30
# speedup vs baseline: 1.0552x; 1.0552x over previous
"""Deformable-DETR encoder (2 layers) fully on 8 Trainium2 NeuronCores.

Sharding: 8 cores = 2 batch x 4 query-bands, one SPMD NEFF for both
layers. Cross-core exchange (full x for each layer's value projection)
is an on-device AllGather within each 4-core replica group.

Per core/layer:
  - value = x @ Wv staged into a zero-padded, transposed bf16 grid
    VT[(head, dh-lane) partition, padded-pos, dh-pair] -- the padding
    implements grid_sample zero-padding for free.
  - per 128-query chunk: off/attn projections (PE, bf16), softmax over
    the 16 (level,point) slots via a block-ones matmul (avoids
    partition reductions), bilinear indices/weights on DVE (exact floor
    via the 2^23 magic-add + is_gt correction), 4 gpsimd ap_gathers
    whose index wrap (s%16 = level*4+point) matches the [(h,lp), q]
    compute layout exactly, per-dh-lane weight replication via a
    stride-0-source DMA bounce through DRAM, fused multiply +
    reduce-over-(level,point) on DVE, then W_out / LN / FFN / LN
    streaming per chunk (residual stream in fp32).

Host I/O is slimmed for the axon tunnel: bf16 band inputs + compact
per-level ref tables expanded on device by broadcast DMA, bf16 output,
weights cached device-side across calls by a persistent jit runner.

kernel(**inputs) takes FULL inputs, returns FULL [2, 13294, 256] fp32.
Retries the device path once, then falls back to a numpy
implementation, so the output is always correct.
"""
import numpy as np

NUM_LAYERS = 2
SHAPES = [(100, 100), (50, 50), (25, 25), (13, 13)]
D, NH, NP, NL = 256, 8, 4, 4
DH = D // NH
DFF = 1024
B = 2
S = sum(h * w for h, w in SHAPES)      # 13294
QB = 3328                               # band width (4 bands; last has 3310 valid)
CH = 128                                # phase-B query chunk
NCHUNK = QB // CH                       # 26
PX, PY = 4, 3                           # pad margins (left/top; right=3, bottom=3)
WP = [w + 7 for (h, w) in SHAPES]       # [107, 57, 32, 20]
HP = [h + 6 for (h, w) in SHAPES]       # [106, 56, 31, 19]
LBASE = [0]
for l in range(1, NL):
    LBASE.append(LBASE[-1] + HP[l - 1] * WP[l - 1])
NPOS = LBASE[-1] + HP[-1] * WP[-1]      # 15906
LSTART = [0, 10000, 12500, 13125]
BANDS = [(b * QB, min((b + 1) * QB, S)) for b in range(4)]
MAGIC = 12582912.0                      # 1.5 * 2^23: fp32 round-to-int trick
f32 = np.float32

_CACHE = {}


def _value_chunks():
    """Row-aligned chunks over S for the value matmul: (qg0, n, level, row0, nrows)."""
    out = []
    for l, (H, W) in enumerate(SHAPES):
        rows_per = max(1, 500 // W)
        r = 0
        while r < H:
            nr = min(rows_per, H - r)
            out.append((LSTART[l] + r * W, nr * W, l, r, nr))
            r += nr
    return out


def _build_nc(stage="full"):
    import concourse.bacc as bacc
    import concourse.mybir as mybir
    from concourse.tile import TileContext
    from concourse.bass_types import AP
    from concourse import library_config

    dt = mybir.dt
    AF = mybir.ActivationFunctionType
    AL = mybir.AluOpType
    AX = mybir.AxisListType

    nc = bacc.Bacc("TRN2", num_devices=8)

    def ext(name, shape, d=dt.float32):
        return nc.dram_tensor(name, shape, d, kind="ExternalInput")

    xband0 = ext("xband0", [QB, D], dt.bfloat16)
    refc = ext("refc", [2, 16, QB])            # (xy, (l,p), q): ref*W-0.5+PAD
    boffp = ext("boffp", [NUM_LAYERS, 2, 128])  # b_off permuted (xy, (h,l,p))
    wvp = ext("wvp", [NUM_LAYERS, D, D], dt.bfloat16)
    bvp = ext("bvp", [NUM_LAYERS, 128, 2])
    wox = ext("wox", [NUM_LAYERS, D, 128], dt.bfloat16)
    woy = ext("woy", [NUM_LAYERS, D, 128], dt.bfloat16)
    wat = ext("wat", [NUM_LAYERS, D, 128], dt.bfloat16)
    bat = ext("bat", [NUM_LAYERS, 128, 1])
    wop = ext("wop", [NUM_LAYERS, D, D], dt.bfloat16)
    bop = ext("bop", [NUM_LAYERS, 128, 2])
    w1 = ext("w1", [NUM_LAYERS, D, DFF], dt.bfloat16)
    b1 = ext("b1", [NUM_LAYERS, 128, 8])
    w2 = ext("w2", [NUM_LAYERS, DFF, D], dt.bfloat16)
    b2 = ext("b2", [NUM_LAYERS, 128, 2])
    g1r = ext("g1r", [NUM_LAYERS, 128, D])
    b1r = ext("b1r", [NUM_LAYERS, 128, D])
    g2r = ext("g2r", [NUM_LAYERS, 128, D])
    b2r = ext("b2r", [NUM_LAYERS, 128, D])
    consts = ext("consts", [128, 8])      # WPL, LBASE, XMAX, YMAX
    identb = ext("identb", [128, 128], dt.bfloat16)
    identf = ext("identf", [128, 128])
    bones = ext("bones", [128, 128], dt.bfloat16)

    x0int = nc.dram_tensor("x0int", [QB, D], dt.bfloat16, kind="Internal")
    x0full = nc.dram_tensor("x0full", [4, QB, D], dt.bfloat16, kind="Internal")
    x1band = nc.dram_tensor("x1band", [QB, D], dt.float32, kind="Internal")
    x1full = nc.dram_tensor("x1full", [4, QB, D], dt.float32, kind="Internal")
    wdram = nc.dram_tensor("wdram", [NUM_LAYERS, NCHUNK, 128, 4 * CH],
                           dt.bfloat16, kind="Internal")
    yband = nc.dram_tensor("yband", [QB, D], dt.bfloat16, kind="ExternalOutput")
    ybq = nc.dram_tensor("ybq", [QB, D], dt.int8, kind="ExternalOutput")
    ysc = nc.dram_tensor("ysc", [QB, 1], dt.float32, kind="ExternalOutput")

    RG = [[0, 1, 2, 3], [4, 5, 6, 7]]
    VCH = _value_chunks()

    with TileContext(nc) as tc:
        with (
            tc.tile_pool(name="persist", bufs=1) as pp,
            tc.tile_pool(name="wts", bufs=1) as wp_,
            tc.tile_pool(name="io", bufs=2) as io,
            tc.tile_pool(name="wk", bufs=1) as wk,
            tc.tile_pool(name="gt", bufs=1) as gt,
            tc.tile_pool(name="ps", bufs=3, space="PSUM") as ps,
            tc.tile_pool(name="psv", bufs=1, space="PSUM") as psv,
            tc.tile_pool(name="pst", bufs=2, space="PSUM") as pstp,
        ):
            nc.gpsimd.load_library(library_config.ap_gather)

            # ---- persistent tiles ----
            vt = pp.tile([128, NPOS * 2], dt.bfloat16, name="vt")
            xtb = pp.tile([128, 2, QB], dt.bfloat16, name="xtb")
            tid_b = pp.tile([128, 128], dt.bfloat16, name="tid_b")
            tid_f = pp.tile([128, 128], dt.float32, name="tid_f")
            tones = pp.tile([128, 128], dt.bfloat16, name="tones")
            tcst = pp.tile([128, 8], dt.float32, name="tcst")
            nc.sync.dma_start(tid_b[:], identb.ap())
            nc.sync.dma_start(tid_f[:], identf.ap())
            nc.sync.dma_start(tones[:], bones.ap())
            nc.sync.dma_start(tcst[:], consts.ap())
            tbofx = pp.tile([128, NUM_LAYERS], dt.float32, name="tbofx")
            tbofy = pp.tile([128, NUM_LAYERS], dt.float32, name="tbofy")
            nc.sync.dma_start(tbofx[:], boffp.ap().rearrange("l a p -> p (l a)")[:, 0::2])
            nc.sync.dma_start(tbofy[:], boffp.ap().rearrange("l a p -> p (l a)")[:, 1::2])
            WPL = tcst[:, 0:1]
            LBC = tcst[:, 1:2]
            XMX = tcst[:, 2:3]
            YMX = tcst[:, 3:4]

            # per-layer weight tiles (reloaded between layers)
            twv = wp_.tile([128, 2, D], dt.bfloat16, name="twv")
            tbv = wp_.tile([128, 2], dt.float32, name="tbv")
            tox = wp_.tile([128, 2, 128], dt.bfloat16, name="tox")
            toy = wp_.tile([128, 2, 128], dt.bfloat16, name="toy")
            tat = wp_.tile([128, 2, 128], dt.bfloat16, name="tat")
            tba = wp_.tile([128, 1], dt.float32, name="tba")
            two = wp_.tile([128, 2, D], dt.bfloat16, name="two")
            tbo = wp_.tile([128, 2], dt.float32, name="tbo")
            tw1 = wp_.tile([128, 2, DFF], dt.bfloat16, name="tw1")
            tb1 = wp_.tile([128, 8], dt.float32, name="tb1")
            tw2 = wp_.tile([128, 8, D], dt.bfloat16, name="tw2")
            tb2 = wp_.tile([128, 2], dt.float32, name="tb2")
            tg1 = wp_.tile([128, D], dt.float32, name="tg1")
            tb1r = wp_.tile([128, D], dt.float32, name="tb1r")
            tg2 = wp_.tile([128, D], dt.float32, name="tg2")
            tb2r = wp_.tile([128, D], dt.float32, name="tb2r")

            def load_layer_weights(li):
                nc.sync.dma_start(twv[:], wvp.ap()[li].rearrange("(k p) m -> p k m", p=128))
                nc.sync.dma_start(tbv[:], bvp.ap()[li])
                nc.sync.dma_start(tox[:], wox.ap()[li].rearrange("(k p) m -> p k m", p=128))
                nc.sync.dma_start(toy[:], woy.ap()[li].rearrange("(k p) m -> p k m", p=128))
                nc.sync.dma_start(tat[:], wat.ap()[li].rearrange("(k p) m -> p k m", p=128))
                nc.sync.dma_start(tba[:], bat.ap()[li])
                nc.sync.dma_start(two[:], wop.ap()[li].rearrange("(k p) m -> p k m", p=128))
                nc.sync.dma_start(tbo[:], bop.ap()[li])
                nc.sync.dma_start(tw1[:], w1.ap()[li].rearrange("(k p) m -> p k m", p=128))
                nc.sync.dma_start(tb1[:], b1.ap()[li])
                nc.sync.dma_start(tw2[:], w2.ap()[li].rearrange("(k p) m -> p k m", p=128))
                nc.sync.dma_start(tb2[:], b2.ap()[li])
                nc.sync.dma_start(tg1[:], g1r.ap()[li])
                nc.sync.dma_start(tb1r[:], b1r.ap()[li])
                nc.sync.dma_start(tg2[:], g2r.ap()[li])
                nc.sync.dma_start(tb2r[:], b2r.ap()[li])

            def transpose_rows_to(dst, src_rows_ap, n, src_bf16=False):
                """src rows [n, 256] (DRAM ap) -> dst [128, 2, nt*128] bf16 cols 0..n."""
                nt = (n + 127) // 128
                if src_bf16:
                    xrb = io.tile([128, nt, D], dt.bfloat16, tag="xrowsb")
                    for t in range(nt):
                        rows = min(128, n - t * 128)
                        nc.sync.dma_start(xrb[:rows, t, :],
                                          src_rows_ap[t * 128:t * 128 + rows, :])
                else:
                    xr = io.tile([128, nt, D], dt.float32, tag="xrows")
                    for t in range(nt):
                        rows = min(128, n - t * 128)
                        nc.sync.dma_start(xr[:rows, t, :],
                                          src_rows_ap[t * 128:t * 128 + rows, :])
                    xrb = io.tile([128, nt, D], dt.bfloat16, tag="xrowsb")
                    nc.vector.tensor_copy(xrb[:], xr[:])
                for t in range(nt):
                    for k in range(2):
                        pst = pstp.tile([128, 128], dt.bfloat16, tag="pt")
                        nc.tensor.transpose(
                            pst[:], xrb[:, t, k * 128:(k + 1) * 128], tid_b[:])
                        nc.scalar.activation(
                            dst[:, k, t * 128:(t + 1) * 128], pst[:], AF.Identity)

            def build_vt(xsrc_ap, src_bf16=False):
                """Value projection into padded transposed bf16 grid."""
                nc.vector.memset(vt[:], 0.0)
                vt3 = vt[:].rearrange("p (n e) -> p n e", e=2)
                for (qg0, n, l, r0, nr) in VCH:
                    W = SHAPES[l][1]
                    xtc = wk.tile([128, 2, 512], dt.bfloat16, tag="xtc")
                    transpose_rows_to(xtc, xsrc_ap[qg0:qg0 + n, :], n, src_bf16)
                    for e in range(2):
                        pv = psv.tile([128, 512], dt.float32, tag="pv")
                        for k in range(2):
                            nc.tensor.matmul(
                                pv[:, :n], twv[:, k, e * 128:(e + 1) * 128],
                                xtc[:, k, :n], start=(k == 0), stop=(k == 1))
                        start = LBASE[l] + (r0 + PY) * WP[l] + PX
                        dst = vt3[:, :, e][:, start:start + nr * WP[l]] \
                            .rearrange("p (r x) -> p r x", x=WP[l])[:, :, :W]
                        nc.scalar.activation(
                            dst, pv[:, :n].rearrange("p (r x) -> p r x", x=W),
                            AF.Identity, bias=tbv[:, e:e + 1])

            def phase_b(li, xband_ap, out_ap, no_gather=False, no_samp=False,
                        no_head=False, in_bf16=False, out_bf16=False):
                """26 query chunks: MSDA + residual/LN + FFN + residual/LN."""
                for c in range(NCHUNK):
                    q0 = c * CH
                    o1 = wk.tile([128, 2, CH], dt.float32, tag="o1")
                    if no_head:
                        nc.vector.memset(o1[:], 0.01)
                    else:
                        macb = wk.tile([128, 2, CH], dt.bfloat16, tag="macb")
                        if no_samp:
                            nc.vector.memset(macb[:], 0.01)
                        else:
                            _msda_chunk(li, c, q0, macb, no_gather=no_gather)
                        # --- W_out projection -> out1T [128, 2, CH] fp32 ---
                        for m in range(2):
                            pw = ps.tile([128, CH], dt.float32, tag="mm")
                            for e in range(2):
                                nc.tensor.matmul(pw[:], two[:, e, m * 128:(m + 1) * 128],
                                                 macb[:, e, :], start=(e == 0), stop=(e == 1))
                            nc.scalar.activation(o1[:, m, :], pw[:], AF.Identity,
                                                 bias=tbo[:, m:m + 1])
                    # --- residual + LN1 (rows [128q, 256]) ---
                    xr = io.tile([128, D],
                                 dt.bfloat16 if in_bf16 else dt.float32,
                                 tag="xrb" if in_bf16 else "xr")
                    nc.sync.dma_start(xr[:], xband_ap[q0:q0 + CH, :])
                    h1 = wk.tile([128, D], dt.float32, tag="h1")
                    for m in range(2):
                        ptr = pstp.tile([128, 128], dt.float32, tag="ptf")
                        nc.tensor.transpose(ptr[:], o1[:, m, :], tid_f[:])
                        nc.vector.tensor_tensor(h1[:, m * 128:(m + 1) * 128],
                                                xr[:, m * 128:(m + 1) * 128],
                                                ptr[:], AL.add)
                    x1c = wk.tile([128, D], dt.float32, tag="x1c")
                    _layernorm(nc, wk, dt, AF, AL, AX, h1, x1c, tg1, tb1r)
                    # --- FFN ---
                    x1b = wk.tile([128, D], dt.bfloat16, tag="x1b")
                    nc.vector.tensor_copy(x1b[:], x1c[:])
                    x1t = wk.tile([128, 2, CH], dt.bfloat16, tag="x1t")
                    for k in range(2):
                        ptb = pstp.tile([128, 128], dt.bfloat16, tag="pt")
                        nc.tensor.transpose(ptb[:], x1b[:, k * 128:(k + 1) * 128], tid_b[:])
                        nc.scalar.activation(x1t[:, k, :], ptb[:], AF.Identity)
                    ht = wk.tile([128, 8, CH], dt.bfloat16, tag="ht")
                    for m in range(8):
                        ph = ps.tile([128, CH], dt.float32, tag="mm")
                        for k in range(2):
                            nc.tensor.matmul(ph[:], tw1[:, k, m * 128:(m + 1) * 128],
                                             x1t[:, k, :], start=(k == 0), stop=(k == 1))
                        nc.scalar.activation(ht[:, m, :], ph[:], AF.Relu,
                                             bias=tb1[:, m:m + 1])
                    y2 = wk.tile([128, 2, CH], dt.float32, tag="y2")
                    for m in range(2):
                        py = ps.tile([128, CH], dt.float32, tag="mm")
                        for k in range(8):
                            nc.tensor.matmul(py[:], tw2[:, k, m * 128:(m + 1) * 128],
                                             ht[:, k, :], start=(k == 0), stop=(k == 7))
                        nc.scalar.activation(y2[:, m, :], py[:], AF.Identity,
                                             bias=tb2[:, m:m + 1])
                    y2b = wk.tile([128, 2, CH], dt.bfloat16, tag="y2b")
                    nc.vector.tensor_copy(y2b[:], y2[:])
                    h2 = wk.tile([128, D], dt.float32, tag="h2")
                    for m in range(2):
                        pt2 = pstp.tile([128, 128], dt.bfloat16, tag="pt")
                        nc.tensor.transpose(pt2[:], y2b[:, m, :], tid_b[:])
                        nc.vector.tensor_tensor(h2[:, m * 128:(m + 1) * 128],
                                                x1c[:, m * 128:(m + 1) * 128],
                                                pt2[:], AL.add)
                    xout = io.tile([128, D], dt.float32, tag="xout")
                    _layernorm(nc, wk, dt, AF, AL, AX, h2, xout, tg2, tb2r)
                    if out_bf16:
                        xob = io.tile([128, D], dt.bfloat16, tag="xob")
                        nc.vector.tensor_copy(xob[:], xout[:])
                        nc.sync.dma_start(out_ap[q0:q0 + CH, :], xob[:])
                        amx = wk.tile([128, 1], dt.float32, tag="amx")
                        nc.vector.tensor_reduce(amx[:], xout[:], AX.X, AL.max,
                                                apply_absolute_value=True)
                        inv = wk.tile([128, 1], dt.float32, tag="ainv")
                        nc.vector.reciprocal(inv[:], amx[:])
                        qf = wk.tile([128, D], dt.float32, tag="qf")
                        nc.vector.tensor_scalar(qf[:], xout[:], inv[:], 127.0,
                                                AL.mult, AL.mult)
                        qi = io.tile([128, D], dt.int8, tag="qi")
                        nc.vector.tensor_copy(qi[:], qf[:])
                        nc.sync.dma_start(ybq.ap()[q0:q0 + CH, :], qi[:])
                        nc.sync.dma_start(ysc.ap()[q0:q0 + CH, :], amx[:])
                    else:
                        nc.sync.dma_start(out_ap[q0:q0 + CH, :], xout[:])

            def _msda_chunk(li, c, q0, macb, no_gather=False):
                # --- projections: offx/offy/attn (PSUM [128, CH]) ---
                pox = ps.tile([128, CH], dt.float32, tag="mm")
                poy = ps.tile([128, CH], dt.float32, tag="mm")
                pat = ps.tile([128, CH], dt.float32, tag="mm")
                for (pt, wt) in ((pox, tox), (poy, toy), (pat, tat)):
                    for k in range(2):
                        nc.tensor.matmul(pt[:], wt[:, k, :],
                                         xtb[:, k, q0:q0 + CH],
                                         start=(k == 0), stop=(k == 1))
                # --- sample coords (padded grid units) ---
                rx = wk.tile([128, CH], dt.float32, tag="rx")
                ry = wk.tile([128, CH], dt.float32, tag="ry")
                for (dst_t, xy) in ((rx, 0), (ry, 1)):
                    base = refc.ap()[xy]
                    rsrc = AP(base.tensor, base.offset + q0,
                              [[0, 8], [QB, 16], [1, CH]])
                    nc.sync.dma_start(dst_t[:], rsrc)
                xg = wk.tile([128, CH], dt.float32, tag="xg")
                yg = wk.tile([128, CH], dt.float32, tag="yg")
                nc.vector.tensor_scalar(xg[:], pox[:], tbofx[:, li:li + 1], None, AL.add)
                nc.vector.tensor_scalar(yg[:], poy[:], tbofy[:, li:li + 1], None, AL.add)
                nc.vector.tensor_tensor(xg[:], xg[:], rx[:], AL.add)
                nc.vector.tensor_tensor(yg[:], yg[:], ry[:], AL.add)
                # --- exact floor + frac ---
                x0 = wk.tile([128, CH], dt.float32, tag="x0")
                y0 = wk.tile([128, CH], dt.float32, tag="y0")
                fx = wk.tile([128, CH], dt.float32, tag="fx")
                fy = wk.tile([128, CH], dt.float32, tag="fy")
                for (g_, o_, f_) in ((xg, x0, fx), (yg, y0, fy)):
                    rnd = wk.tile([128, CH], dt.float32, tag="rnd")
                    nc.vector.tensor_scalar(rnd[:], g_[:], MAGIC, -MAGIC,
                                            AL.add, AL.add)
                    msk = wk.tile([128, CH], dt.float32, tag="msk")
                    nc.vector.tensor_tensor(msk[:], rnd[:], g_[:], AL.is_gt)
                    nc.vector.tensor_tensor(o_[:], rnd[:], msk[:], AL.subtract)
                    nc.vector.tensor_tensor(f_[:], g_[:], o_[:], AL.subtract)
                # clamp (keeps +1 corners inside each level block)
                nc.vector.tensor_scalar(x0[:], x0[:], 0.0, XMX, AL.max, AL.min)
                nc.vector.tensor_scalar(y0[:], y0[:], 0.0, YMX, AL.max, AL.min)
                # --- linear indices, 4 corners, int16 ---
                ib = wk.tile([128, CH], dt.float32, tag="ib")
                nc.vector.tensor_scalar(ib[:], y0[:], WPL, LBC, AL.mult, AL.add)
                nc.vector.tensor_tensor(ib[:], ib[:], x0[:], AL.add)
                ir = wk.tile([128, CH], dt.float32, tag="ir")
                nc.vector.tensor_scalar(ir[:], ib[:], WPL, None, AL.add)
                idx = []
                for (src_, off) in ((ib, 0.0), (ib, 1.0), (ir, 0.0), (ir, 1.0)):
                    ii = wk.tile([128, CH], dt.int16, tag=f"idx{len(idx)}")
                    if off == 0.0:
                        nc.vector.tensor_copy(ii[:], src_[:])
                    else:
                        tmp = wk.tile([128, CH], dt.float32, tag="itmp")
                        nc.vector.tensor_scalar(tmp[:], src_[:], off, None, AL.add)
                        nc.vector.tensor_copy(ii[:], tmp[:])
                    idx.append(ii)
                # --- softmax over 16 (l,p) per head ---
                ex = wk.tile([128, CH], dt.bfloat16, tag="ex")
                nc.scalar.activation(ex[:], pat[:], AF.Exp, bias=tba[:, 0:1])
                pse = ps.tile([128, CH], dt.float32, tag="mm")
                nc.tensor.matmul(pse[:], tones[:], ex[:], start=True, stop=True)
                rs = wk.tile([128, CH], dt.float32, tag="rs")
                nc.vector.reciprocal(rs[:], pse[:])
                aw = wk.tile([128, CH], dt.float32, tag="aw")
                nc.vector.tensor_tensor(aw[:], ex[:], rs[:], AL.mult)
                # --- corner weights -> wstack bf16 [128, 4, CH] ---
                wx0 = wk.tile([128, CH], dt.float32, tag="wx0")
                wy0 = wk.tile([128, CH], dt.float32, tag="wy0")
                nc.vector.tensor_scalar(wx0[:], fx[:], -1.0, 1.0, AL.mult, AL.add)
                nc.vector.tensor_scalar(wy0[:], fy[:], -1.0, 1.0, AL.mult, AL.add)
                u0 = wk.tile([128, CH], dt.float32, tag="u0")
                u1 = wk.tile([128, CH], dt.float32, tag="u1")
                nc.vector.tensor_tensor(u0[:], aw[:], wx0[:], AL.mult)
                nc.vector.tensor_tensor(u1[:], aw[:], fx[:], AL.mult)
                wst = wk.tile([128, 4, CH], dt.bfloat16, tag="wst")
                nc.vector.tensor_tensor(wst[:, 0, :], u0[:], wy0[:], AL.mult)
                nc.vector.tensor_tensor(wst[:, 1, :], u1[:], wy0[:], AL.mult)
                nc.vector.tensor_tensor(wst[:, 2, :], u0[:], fy[:], AL.mult)
                nc.vector.tensor_tensor(wst[:, 3, :], u1[:], fy[:], AL.mult)
                # --- replicate weights across 16 dh-lanes via DRAM bounce ---
                wslot = wdram.ap()[li][c]
                nc.sync.dma_start(wslot, wst[:].rearrange("p a q -> p (a q)"))
                wrep = wk.tile([128, 16, 4 * CH], dt.bfloat16, tag="wrep")
                rep_src = AP(wslot.tensor, wslot.offset,
                             [[16 * 4 * CH, 8], [0, 16], [4 * CH, 16],
                              [1, 4 * CH]])
                nc.sync.dma_start(wrep[:], rep_src)
                # --- 4 gathers + weighted reduce over (lp, corners) ---
                red = wk.tile([128, 8, CH], dt.float32, tag="red")
                for ci in range(4):
                    g = gt.tile([128, CH * 16, 2], dt.bfloat16, tag=f"g{ci}")
                    if no_gather:
                        nc.vector.memset(g[:], 0.25)
                    else:
                        nc.gpsimd.ap_gather(g[:], vt[:], idx[ci][:], channels=128,
                                            num_elems=NPOS, d=2, num_idxs=CH * 16)
                    prod = wk.tile([128, CH, 2, 16], dt.bfloat16, tag="prod")
                    w3 = wrep[:, :, ci * CH:(ci + 1) * CH] \
                        .rearrange("p l q -> p q l")
                    w4 = AP(w3.tensor, w3.offset,
                            [list(w3.ap[0]), list(w3.ap[1]), [0, 2],
                             list(w3.ap[2])])
                    nc.vector.tensor_tensor(
                        prod[:],
                        g[:].rearrange("p (q l) e -> p q e l", l=16),
                        w4, AL.mult)
                    nc.vector.tensor_reduce(
                        red[:, 2 * ci:2 * ci + 2, :].rearrange("p e q -> p q e"),
                        prod[:], AX.X, AL.add)
                mac = wk.tile([128, 2, CH], dt.float32, tag="mac")
                nc.vector.tensor_tensor(mac[:], red[:, 0:2, :], red[:, 2:4, :], AL.add)
                nc.vector.tensor_tensor(mac[:], mac[:], red[:, 4:6, :], AL.add)
                nc.vector.tensor_tensor(mac[:], mac[:], red[:, 6:8, :], AL.add)
                nc.vector.tensor_copy(macb[:], mac[:])

            # ================= stage x0 + exchange =================
            for c in range(NCHUNK // 2):
                r0, r1 = 2 * c * CH, 2 * (c + 1) * CH
                stg = io.tile([128, 2, D], dt.bfloat16, tag="xstage")
                nc.sync.dma_start(stg[:], xband0.ap()[r0:r1, :]
                                  .rearrange("(t p) d -> p t d", p=128))
                nc.sync.dma_start(x0int.ap()[r0:r1, :]
                                  .rearrange("(t p) d -> p t d", p=128), stg[:])
            nc.gpsimd.collective_compute(
                "AllGather", mybir.AluOpType.bypass, RG,
                ins=[x0int.ap()], outs=[x0full.ap()])

            # ================= layer 0 =================
            load_layer_weights(0)
            build_vt(x0full.ap().rearrange("b q d -> (b q) d"), src_bf16=True)
            for c in range(NCHUNK):
                xtc2 = wk.tile([128, 2, 512], dt.bfloat16, tag="xtc")
                transpose_rows_to(xtc2, xband0.ap()[c * CH:(c + 1) * CH, :], CH,
                                  src_bf16=True)
                nc.vector.tensor_copy(xtb[:, :, c * CH:(c + 1) * CH], xtc2[:, :, :CH])
            if stage == "a":
                dump = wk.tile([128, 2048], dt.float32, tag="dump")
                nc.vector.tensor_copy(dump[:], vt[:, :2048])
                nc.sync.dma_start(
                    yband.ap().rearrange("q d -> (q d)")[0:262144]
                    .rearrange("(p f) -> p f", p=128), dump[:])
            else:
                phase_b(0, xband0.ap(), x1band.ap() if stage == "full" else yband.ap(),
                        no_gather=(stage == "bng"),
                        no_samp=(stage == "b_mid"),
                        no_head=(stage == "b_tail"), in_bf16=True,
                        out_bf16=(stage != "full"))

            if stage == "full":
                # ================= exchange =================
                nc.gpsimd.collective_compute(
                    "AllGather", mybir.AluOpType.bypass, RG,
                    ins=[x1band.ap()], outs=[x1full.ap()])

                # ================= layer 1 =================
                load_layer_weights(1)
                build_vt(x1full.ap().rearrange("b q d -> (b q) d"))
                for c in range(NCHUNK):
                    xtc3 = wk.tile([128, 2, 512], dt.bfloat16, tag="xtc")
                    transpose_rows_to(xtc3, x1band.ap()[c * CH:(c + 1) * CH, :], CH)
                    nc.vector.tensor_copy(xtb[:, :, c * CH:(c + 1) * CH], xtc3[:, :, :CH])
                phase_b(1, x1band.ap(), yband.ap(), out_bf16=True)

    nc.finalize()
    return nc


def _layernorm(nc, wk, dt, AF, AL, AX, hin, hout, gtile, btile):
    """Row layernorm [128 tokens, 256], eps=1e-5, with replicated g/b tiles."""
    sm = wk.tile([128, 1], dt.float32, tag="ln_sm")
    nc.vector.tensor_reduce(sm[:], hin[:], AX.X, AL.add)
    scr = wk.tile([128, 256], dt.float32, tag="ln_scr")
    nc.vector.tensor_tensor(scr[:], hin[:], hin[:], AL.mult)
    sq = wk.tile([128, 1], dt.float32, tag="ln_sq")
    nc.vector.tensor_reduce(sq[:], scr[:], AX.X, AL.add)
    nc.vector.tensor_scalar(sq[:], sq[:], 1.0 / 256, None, AL.mult)
    m = wk.tile([128, 1], dt.float32, tag="ln_m")
    nc.vector.tensor_scalar(m[:], sm[:], 1.0 / 256, None, AL.mult)
    mm2 = wk.tile([128, 1], dt.float32, tag="ln_mm")
    nc.vector.tensor_tensor(mm2[:], m[:], m[:], AL.mult)
    var = wk.tile([128, 1], dt.float32, tag="ln_v")
    nc.vector.tensor_tensor(var[:], sq[:], mm2[:], AL.subtract)
    nc.vector.tensor_scalar(var[:], var[:], 1e-5, None, AL.add)
    std = wk.tile([128, 1], dt.float32, tag="ln_s")
    nc.scalar.activation(std[:], var[:], AF.Sqrt)
    rstd = wk.tile([128, 1], dt.float32, tag="ln_r")
    nc.vector.reciprocal(rstd[:], std[:])
    xh = wk.tile([128, 256], dt.float32, tag="ln_xh")
    nc.vector.tensor_scalar(xh[:], hin[:], m[:], rstd[:], AL.subtract, AL.mult)
    nc.vector.tensor_tensor(xh[:], xh[:], gtile[:], AL.mult)
    nc.vector.tensor_tensor(hout[:], xh[:], btile[:], AL.add)


def _host_prep(src, spatial_shapes, valid_ratios, W_off, b_off, W_attn, b_attn,
               W_val, b_val, W_out, b_out, ln1_g, ln1_b, W1, b1, W2, b2,
               ln2_g, ln2_b):
    """Build per-core in_maps (weights permuted to device layouts)."""
    import ml_dtypes
    bf = ml_dtypes.bfloat16
    L = NUM_LAYERS

    # reference points (exact reference formula, incl. valid_ratios)
    vr = np.asarray(valid_ratios, f32)           # [B, NL, 2]
    refs = []
    for lvl, (H_, W_) in enumerate(SHAPES):
        ry, rx = np.meshgrid(np.linspace(0.5, H_ - 0.5, H_, dtype=f32),
                             np.linspace(0.5, W_ - 0.5, W_, dtype=f32),
                             indexing='ij')
        ry = ry.reshape(-1)[None] / (vr[:, None, lvl, 1] * H_)
        rx = rx.reshape(-1)[None] / (vr[:, None, lvl, 0] * W_)
        refs.append(np.stack([rx, ry], -1))
    ref = np.concatenate(refs, 1)                 # [B, S, 2]
    refl = ref[:, :, None] * vr[:, None]          # [B, S, NL, 2]

    # partition maps: p = 16h + 4l + pp
    hh = np.arange(128) // 16
    ll = (np.arange(128) % 16) // 4
    pp_ = np.arange(128) % 4
    Wl = np.array([SHAPES[l][1] for l in range(NL)], f32)
    Hl = np.array([SHAPES[l][0] for l in range(NL)], f32)

    # per (layer, batch): REFB tiles [128, QB] per band
    off_cols_x = ((hh * NL + ll) * NP + pp_) * 2
    off_cols_y = off_cols_x + 1
    # compact per-core ref tiles [2(xy), NL, QB] and permuted offset biases
    refc_all = {}
    for bi in range(B):
        gx = refl[bi, :, :, 0] * Wl[None, :] - 0.5 + PX    # [S, NL]
        gy = refl[bi, :, :, 1] * Hl[None, :] - 0.5 + PY
        for bd in range(4):
            a, bnd = BANDS[bd]
            t = np.full((2, 16, QB), 10.0, f32)
            t[0, :, :bnd - a] = np.repeat(gx[a:bnd].T, 4, axis=0)
            t[1, :, :bnd - a] = np.repeat(gy[a:bnd].T, 4, axis=0)
            refc_all[(bi, bd)] = t
    boffp = np.zeros((L, 2, 128), f32)
    for li in range(L):
        boffp[li, 0] = np.asarray(b_off[li], f32)[off_cols_x]
        boffp[li, 1] = np.asarray(b_off[li], f32)[off_cols_y]

    # weight permutations (same for every core)
    j16 = np.arange(128) % 16
    h8 = np.arange(128) // 16
    wvp = np.zeros((L, D, D), bf)
    bvp = np.zeros((L, 128, 2), f32)
    wop = np.zeros((L, D, D), bf)
    for li in range(L):
        for e in range(2):
            cols = h8 * 32 + 16 * e + j16          # dh for partition (h,j), plane e
            wvp[li, :, e * 128:(e + 1) * 128] = np.asarray(W_val[li], f32)[:, cols].astype(bf)
            bvp[li, :, e] = np.asarray(b_val[li], f32)[cols]
            wop[li, e * 128:(e + 1) * 128, :] = np.asarray(W_out[li], f32)[cols, :].astype(bf)
    wox = np.stack([np.asarray(W_off[li], f32)[:, off_cols_x].astype(bf) for li in range(L)])
    woy = np.stack([np.asarray(W_off[li], f32)[:, off_cols_y].astype(bf) for li in range(L)])
    wat = np.stack([np.asarray(W_attn[li], f32).astype(bf) for li in range(L)])
    bat = np.stack([np.asarray(b_attn[li], f32)[:, None] for li in range(L)])
    bop = np.stack([np.asarray(b_out[li], f32).reshape(2, 128).T for li in range(L)])
    w1s = np.stack([np.asarray(W1[li], f32).astype(bf) for li in range(L)])
    b1s = np.stack([np.asarray(b1[li], f32).reshape(8, 128).T for li in range(L)])
    w2s = np.stack([np.asarray(W2[li], f32).astype(bf) for li in range(L)])
    b2s = np.stack([np.asarray(b2[li], f32).reshape(2, 128).T for li in range(L)])
    g1r = np.stack([np.tile(np.asarray(ln1_g[li], f32), (128, 1)) for li in range(L)])
    b1r = np.stack([np.tile(np.asarray(ln1_b[li], f32), (128, 1)) for li in range(L)])
    g2r = np.stack([np.tile(np.asarray(ln2_g[li], f32), (128, 1)) for li in range(L)])
    b2r = np.stack([np.tile(np.asarray(ln2_b[li], f32), (128, 1)) for li in range(L)])

    consts = np.zeros((128, 8), f32)
    consts[:, 0] = np.array(WP, f32)[ll]
    consts[:, 1] = np.array(LBASE, f32)[ll]
    consts[:, 2] = np.array(WP, f32)[ll] - 2
    consts[:, 3] = np.array(HP, f32)[ll] - 2
    identb = np.eye(128, dtype=bf)
    identf = np.eye(128, dtype=f32)
    bones = (np.arange(128)[:, None] // 16 == np.arange(128)[None, :] // 16).astype(bf)

    srcf = np.asarray(src, f32)
    in_maps = []
    for core in range(8):
        bi, bd = core // 4, core % 4
        a, bnd = BANDS[bd]
        xband = np.zeros((QB, D), bf)
        xband[:bnd - a] = srcf[bi, a:bnd].astype(bf)
        in_maps.append({
            "xband0": xband,
            "refc": refc_all[(bi, bd)], "boffp": boffp,
            "wvp": wvp, "bvp": bvp, "wox": wox, "woy": woy, "wat": wat,
            "bat": bat, "wop": wop, "bop": bop, "w1": w1s, "b1": b1s,
            "w2": w2s, "b2": b2s, "g1r": g1r, "b1r": b1r, "g2r": g2r,
            "b2r": b2r, "consts": consts, "identb": identb,
            "identf": identf, "bones": bones,
        })
    return in_maps


class _Runner:
    """Persistent jit wrapper around the bass NEFF (trace once, reuse)."""

    def __init__(self, nc, n_cores=8):
        import jax
        import concourse.mybir as mybir
        from concourse import bass2jax
        from jax.sharding import Mesh, PartitionSpec
        from jax.experimental.shard_map import shard_map

        bass2jax.install_neuronx_cc_hook()
        self.n_cores = n_cores
        partition_name = (nc.partition_id_tensor.name
                          if nc.partition_id_tensor else None)
        in_names, out_names, out_avals, zero_shapes = [], [], [], []
        for alloc in nc.m.functions[0].allocations:
            if not isinstance(alloc, mybir.MemoryLocationSet):
                continue
            name = alloc.memorylocations[0].name
            if alloc.kind == "ExternalInput":
                if name != partition_name:
                    in_names.append(name)
            elif alloc.kind == "ExternalOutput":
                shape = tuple(alloc.tensor_shape)
                dtype = mybir.dt.np(alloc.dtype)
                out_names.append(name)
                out_avals.append(jax.core.ShapedArray(shape, dtype))
                zero_shapes.append((shape, dtype))
        self.in_names = list(in_names)
        self.out_names = out_names
        self.out_avals = out_avals
        self.zero_shapes = zero_shapes
        n_params = len(in_names)
        donate = ()
        all_names = in_names + out_names
        if partition_name is not None:
            all_names.append(partition_name)

        def _body(*args):
            operands = list(args)
            if partition_name is not None:
                operands.append(bass2jax.partition_id_tensor())
            outs = bass2jax._bass_exec_p.bind(
                *operands, out_avals=tuple(out_avals),
                in_names=tuple(all_names), out_names=tuple(out_names),
                lowering_input_output_aliases=(),
                sim_require_finite=True, sim_require_nnan=True, nc=nc)
            return tuple(outs)

        devices = jax.devices()[:n_cores]
        mesh = Mesh(np.asarray(devices), ("core",))
        self.sharding = jax.sharding.NamedSharding(mesh, PartitionSpec("core"))
        in_specs = (PartitionSpec("core"),) * (n_params + len(out_names))
        out_specs = (PartitionSpec("core"),) * len(out_names)
        self.jf = jax.jit(
            shard_map(_body, mesh=mesh, in_specs=in_specs,
                      out_specs=out_specs, check_rep=False),
            donate_argnums=donate, keep_unused=True)
        import jax.numpy as jnp

        def _mkzeros():
            return tuple(jnp.zeros((n_cores * s[0], *s[1:]), d)
                         for (s, d) in self.zero_shapes)
        self.zf = jax.jit(_mkzeros,
                          out_shardings=(self.sharding,) * len(out_names))
        self._zeros = None
        self._dev_cache = {}

    def __call__(self, in_maps):
        import jax
        n = self.n_cores
        concat_in = []
        for name in self.in_names:
            arrs = [np.asarray(in_maps[c][name]) for c in range(n)]
            key = tuple(id(a) for a in arrs)
            hit = self._dev_cache.get(name)
            if hit is not None and hit[0] == key:
                concat_in.append(hit[1])
            else:
                dev = jax.device_put(np.concatenate(arrs, axis=0),
                                     self.sharding)
                self._dev_cache[name] = (key, dev)
                concat_in.append(dev)
        if self._zeros is None:
            self._zeros = self.zf()
        out_arrs = self.jf(*concat_in, *self._zeros)
        jax.block_until_ready(out_arrs)
        fetch = getattr(self, "fetch_names", None) or self.out_names
        take = [(i, name) for i, name in enumerate(self.out_names)
                if name in fetch]
        from concurrent.futures import ThreadPoolExecutor
        with ThreadPoolExecutor(max(1, len(take))) as ex:
            arrs = list(ex.map(lambda t: np.asarray(out_arrs[t[0]]), take))
        res = [dict() for _ in range(n)]
        for (i, name), arr in zip(take, arrs):
            arr = arr.reshape(n, *self.out_avals[i].shape)
            for c in range(n):
                res[c][name] = arr[c]
        return res


USE_INT8_OUT = True


def run_device(in_maps):
    if "runner" not in _CACHE:
        if "nc" not in _CACHE:
            _CACHE["nc"] = _build_nc()
        _CACHE["runner"] = _Runner(_CACHE["nc"])
        _CACHE["runner"].fetch_names = (
            ["ybq", "ysc"] if USE_INT8_OUT else ["yband"])
    results = _CACHE["runner"](in_maps)
    out = np.zeros((B, S, D), f32)
    for core in range(8):
        bi, bd = core // 4, core % 4
        a, bnd = BANDS[bd]
        r = results[core]
        if USE_INT8_OUT:
            q = r["ybq"][:bnd - a].astype(f32)
            s = r["ysc"][:bnd - a].astype(f32) * (1.0 / 127.0)
            out[bi, a:bnd] = q * s
        else:
            out[bi, a:bnd] = r["yband"][:bnd - a].astype(f32)
    return out


# ---------------- numpy fallback (correctness insurance) ----------------

def _np_layer_norm(x, g, b, eps=1e-5):
    m = x.mean(-1, keepdims=True, dtype=f32)
    v = x.var(-1, keepdims=True)
    return ((x - m) / np.sqrt(v + eps) * g + b).astype(f32)


def _np_softmax(x):
    m = x.max(-1, keepdims=True)
    e = np.exp(x - m)
    return (e / e.sum(-1, keepdims=True)).astype(f32)


def _np_msda(x, refl, Wv, bv, Wo, bo, Wa, ba, Wout, bout):
    value = (x @ Wv + bv).reshape(S, NH, DH)
    off = (x @ Wo + bo).reshape(S, NH, NL, NP, 2)
    attn = _np_softmax((x @ Wa + ba).reshape(S, NH, NL * NP)).reshape(S, NH, NL, NP)
    h_br = np.arange(NH, dtype=np.int32)[None, :, None]
    out = np.zeros((S, NH, DH), f32)
    start = 0
    PAD = 4
    for l, (H_, W_) in enumerate(SHAPES):
        Hp_, Wp_ = H_ + 2 * PAD, W_ + 2 * PAD
        vp = np.zeros((Hp_, Wp_, NH, DH), f32)
        vp[PAD:PAD + H_, PAD:PAD + W_] = value[start:start + H_ * W_].reshape(H_, W_, NH, DH)
        vp = vp.reshape(Hp_ * Wp_, NH, DH)
        xg_ = refl[:, l, 0][:, None, None] * W_ - 0.5 + off[:, :, l, :, 0] + PAD
        yg_ = refl[:, l, 1][:, None, None] * H_ - 0.5 + off[:, :, l, :, 1] + PAD
        x0 = np.floor(xg_)
        y0 = np.floor(yg_)
        fx = (xg_ - x0).astype(f32)
        fy = (yg_ - y0).astype(f32)
        i0 = (np.clip(y0, 0, Hp_ - 2) * Wp_ + np.clip(x0, 0, Wp_ - 2)).astype(np.int32)
        a_l = attn[:, :, l]
        for didx, w in ((0, (1 - fx) * (1 - fy)), (1, fx * (1 - fy)),
                        (Wp_, (1 - fx) * fy), (Wp_ + 1, fx * fy)):
            g = vp[i0 + didx, h_br]
            out += np.einsum('qhpd,qhp->qhd', g, (w * a_l).astype(f32))
        start += H_ * W_
    return (out.reshape(S, D) @ Wout + bout).astype(f32)


def _np_kernel(src, valid_ratios, W_off, b_off, W_attn, b_attn, W_val, b_val,
               W_out, b_out, ln1_g, ln1_b, W1, b1, W2, b2, ln2_g, ln2_b):
    vr = np.asarray(valid_ratios, f32)
    refs = []
    for lvl, (H_, W_) in enumerate(SHAPES):
        ry, rx = np.meshgrid(np.linspace(0.5, H_ - 0.5, H_, dtype=f32),
                             np.linspace(0.5, W_ - 0.5, W_, dtype=f32), indexing='ij')
        ry = ry.reshape(-1)[None] / (vr[:, None, lvl, 1] * H_)
        rx = rx.reshape(-1)[None] / (vr[:, None, lvl, 0] * W_)
        refs.append(np.stack([rx, ry], -1))
    ref = np.concatenate(refs, 1)
    refl = ref[:, :, None] * vr[:, None]
    x = np.asarray(src, f32).copy()
    for i in range(NUM_LAYERS):
        for bi in range(B):
            x2 = _np_msda(x[bi], refl[bi], W_val[i], b_val[i], W_off[i], b_off[i],
                          W_attn[i], b_attn[i], W_out[i], b_out[i])
            xb = _np_layer_norm(x[bi] + x2, ln1_g[i], ln1_b[i])
            h = np.maximum(xb @ W1[i] + b1[i], 0) @ W2[i] + b2[i]
            x[bi] = _np_layer_norm(xb + h, ln2_g[i], ln2_b[i])
    return x.astype(f32)


def kernel(src, spatial_shapes, valid_ratios, W_off, b_off, W_attn, b_attn,
           W_val, b_val, W_out, b_out, ln1_g, ln1_b, W1, b1, W2, b2,
           ln2_g, ln2_b):
    args = dict(src=src, spatial_shapes=spatial_shapes, valid_ratios=valid_ratios,
                W_off=W_off, b_off=b_off, W_attn=W_attn, b_attn=b_attn,
                W_val=W_val, b_val=b_val, W_out=W_out, b_out=b_out,
                ln1_g=ln1_g, ln1_b=ln1_b, W1=W1, b1=b1, W2=W2, b2=b2,
                ln2_g=ln2_g, ln2_b=ln2_b)
    try:
        in_maps = _host_prep(**args)
    except Exception:
        import traceback
        traceback.print_exc()
        a2 = dict(args)
        a2.pop("spatial_shapes")
        return _np_kernel(**a2)
    for attempt in range(2):
        try:
            return run_device(in_maps)
        except Exception:
            import traceback
            traceback.print_exc()
            _CACHE.pop("runner", None)
    a2 = dict(args)
    a2.pop("spatial_shapes")
    return _np_kernel(**a2)
